# revision 40
# baseline (speedup 1.0000x reference)
"""Trainium2 Bass kernel: separable parabolic morphological dilation (11-tap).

nn_Dilation2dSingle: im [8, 32, 512, 512] f32, se_coef scalar, se [11, 1].
    bias[k] = se_coef * se[k, 0] = a * k^2,  a = se_coef / 4
    out = vdilate(hdilate(im)) with NEG=-10000 padding.

The 11-tap parabolic window is decomposed exactly into a cascade of five
3-tap max-plus stages with biases c_m = a*(2m-1) (partial sums a*k^2):
    u_m[j] = max(u_{m-1}[j], max(u_{m-1}[j-1], u_{m-1}[j+1]) - c_m)
Each stage runs as ONE custom DVE instruction (DIL3_ANT): a hand-built
uop program recovers the center tap as a one-element temporal delay of
SRC_0 inside the 8-slice pipeline (block 0 captures its previous-cycle
ALU flop into a delay lane), so a stage costs 1 elem/cycle instead of
the 2 ops/elem of the stock tensor_max + scalar_tensor_tensor pair.
DVE work: 5 passes/axis instead of 10 -> ~2x.

Each DIL3 stream's first element (and each segment-boundary first
element) computes with a stale delayed tap. Stage m's garbage cell sits
at position m and is only ever read by stage m+1's own first element --
itself garbage -- so the junk chain is self-contained; the left halo is
widened to 6 so the final stage's garbage lands in a never-read leading
column of the accumulator tile.

Two row/col-tiles are batched per DVE instruction (8 segments per 3D
access pattern) to halve instruction-dispatch overhead; intermediates
are fp16 to fit the batched tiles in SBUF (NEG is exactly representable
in fp16; ~5e-4 relative rounding vs the 2e-2 gate). Input/output stay
fp32.

Sharding: pure data-parallel over batch (8 cores x 1 batch each); no
collectives.
"""

from contextlib import ExitStack
from dataclasses import dataclass

import numpy as np

import concourse.bacc as bacc
import concourse.dve_ops as dve_ops_mod
import concourse.mybir as mybir
import concourse.tile as tile
from concourse.bass_utils import run_bass_kernel_spmd
from concourse.dve_ops import DveOp
from concourse.dve_spec import C0, Spec, Src0, Src1, maxx
from concourse.dve_uop import (
    ENABLE,
    AluInp,
    AluOp,
    DelayInp,
    DveOpSpec,
    InpSel,
    OutPath,
    OutSel,
    Trigger,
    UopConfig,
)

F32 = mybir.dt.float32
F16 = mybir.dt.float16
NEG = -10000.0
R = 5  # dilation radius (window 11)
EL, ER = 6, 5  # left/right halo widths (left widened for the junk cell)
JW = 1  # junk leading column on the accumulator tiles

# Hardcoded problem shape (per spec).
B, C, H, W = 8, 32, 512, 512
N_CORES = 8
CP = 4  # channels per group
TB = 2  # row/col tiles batched per DVE instruction
NS = CP * TB  # segments per DVE instruction


# --- DIL3_ANT custom DVE op ------------------------------------------------ #


def _build_dil3_uops() -> list[UopConfig]:
    """out[j] = max(x[j], max(x[j-1], x[j+1]) - c) with in0 = x[j+1],
    in1 = x[j-1]; the center x[j] is block 0's previous-cycle flop."""
    u = UopConfig()
    u.enable_input(InpSel.SRC_0, 1)  # delay_0 <- x[j+1]
    u.enable_input(InpSel.SRC_1, 2)  # delay_1 <- x[j-1]
    u.enable_input(InpSel.CONST_0, 3)  # delay_2 <- c
    dp = u.datapath_config
    dp[0].enable_alu(AluOp.BYPASS, AluInp.PREV_DELAY_0)
    dp[0].enable_delay_from_src(DelayInp.CURR_ALU_OUT, 3)  # delay_3 <- x[j]
    dp[0].pass_through_delay(1, 2)
    dp[1].enable_alu(AluOp.MAX, AluInp.PREV_ALU_OUT, AluInp.PREV_DELAY_1)
    dp[1].pass_through_delay(2, 3)
    dp[2].enable_alu(AluOp.SUBTRACT, AluInp.PREV_ALU_OUT, AluInp.PREV_DELAY_2)
    dp[2].pass_through_delay(3)
    dp[3].enable_alu(AluOp.MAX, AluInp.PREV_ALU_OUT, AluInp.PREV_DELAY_3)
    for k in range(4, 8):
        dp[k].pass_through_alu()
    u.require_inp0 = ENABLE
    u.require_inp1 = ENABLE
    u.trigger = (Trigger.SRC_TENSOR_DONE, Trigger.NONE, Trigger.NONE)
    u.enable_output(OutSel.ALU_OUT, OutPath.WR0_LO)
    return [u]


_HAND_CACHE: dict = {}


@dataclass(frozen=True)
class HandDveOp(DveOp):
    """DveOp whose table program is hand-built rather than lowered from
    `spec`; `spec` is only the structural stand-in for _custom_dve."""

    def compile(self, ver):
        key = (self.name, ver)
        if key not in _HAND_CACHE:
            s = DveOpSpec(
                name=self.name,
                opcode=dve_ops_mod.get_dve_sub_opcode(self.name),
                uops=_build_dil3_uops(),
                rd1_en=True,
            )
            s.validate(ver)
            _HAND_CACHE[key] = s
        return _HAND_CACHE[key]


def _dil3_ref(in0, in1, s0, s1, imm2):
    return np.maximum(in1, in0 - s0).astype(np.float32)


DIL3 = HandDveOp(
    "DIL3_ANT",
    Spec(body=maxx(Src1, Src0 - C0), reference=_dil3_ref),
    subdim=False,
    uops_sha={},
)


def register_dil3() -> None:
    if DIL3.name in dve_ops_mod._SUB_OPCODE_FOR_NAME:
        return
    row = dve_ops_mod._CUSTOM_DVE_ROW_BASE + len(dve_ops_mod.OPS)
    assert row < 0x20, f"no free custom-DVE row for {DIL3.name}"
    dve_ops_mod.OPS.append(DIL3)
    dve_ops_mod._SUB_OPCODE_FOR_NAME[DIL3.name] = row
    dve_ops_mod.CUSTOM_DVE_SPECS[DIL3.name] = DIL3.spec


register_dil3()


# --- kernel ---------------------------------------------------------------- #


def _cascade(nc, mid_pool, seg3, bias_t, S, L, acc3, tag="mid", bufs=None):
    """Five DIL3 stages along the innermost axis of seg3 [128, nseg, S]
    (positions: [0,EL) NEG pad, [EL,EL+L) payload, [EL+L,S) NEG pad).
    Writes acc3 [128, nseg, L+JW]; valid payload at [JW, JW+L).
    Intermediates are fp16 (values +-~30; NEG is exact in fp16; the
    ~5e-4 relative rounding is far inside the 2e-2 gate)."""
    prev = seg3
    for m in range(1, R):
        kw = {} if bufs is None else {"bufs": bufs}
        mid = mid_pool.tile([128, seg3.shape[1] * S], F16, tag=tag, **kw)
        midv = mid[:].rearrange("p (s c) -> p s c", s=seg3.shape[1])
        nc.vector._custom_dve(
            DIL3,
            out=midv[:, :, m : S - m],
            in0=prev[:, :, m + 1 : S - m + 1],
            in1=prev[:, :, m - 1 : S - m - 1],
            s0=bias_t[:, m - 1 : m],
        )
        prev = midv
    nc.vector._custom_dve(
        DIL3,
        out=acc3[:, :, 0 : L + JW],
        in0=prev[:, :, R + 1 : S - R + 1],
        in1=prev[:, :, R - 1 : S - R - 1],
        s0=bias_t[:, R - 1 : R],
    )


def build_nc(C=C, H=H, W=W, CP=CP, reps=1):
    assert H % 128 == 0 and W % 128 == 0 and C % CP == 0
    nH, nW, nG = H // 128, W // 128, C // CP
    nHB, nWB = nH // TB, nW // TB  # batched tile counts
    SW, SH = W + EL + ER, H + EL + ER
    AW, AH = W + JW, H + JW  # accumulator widths per segment

    nc = bacc.Bacc("TRN2", target_bir_lowering=False, debug=False)
    im = nc.dram_tensor("im", [C, H, W], F32, kind="ExternalInput")
    bias = nc.dram_tensor("bias5", [128, R], F32, kind="ExternalInput")
    iden = nc.dram_tensor("iden", [128, 128], F16, kind="ExternalInput")
    out = nc.dram_tensor("out", [C, H, W], F32, kind="ExternalOutput")

    with tile.TileContext(nc) as tc, ExitStack() as ctx:
        const_pool = ctx.enter_context(tc.tile_pool(name="const", bufs=1))
        hin_pool = ctx.enter_context(tc.tile_pool(name="hin", bufs=3))
        hmid_pool = ctx.enter_context(tc.tile_pool(name="hmid", bufs=2))
        hacc_pool = ctx.enter_context(tc.tile_pool(name="hacc", bufs=2 * nHB + 1))
        vin_pool = ctx.enter_context(tc.tile_pool(name="vin", bufs=3))
        vmid_pool = ctx.enter_context(tc.tile_pool(name="vmid", bufs=3))
        vacc_pool = ctx.enter_context(tc.tile_pool(name="vacc", bufs=nWB + 1))
        st_pool = ctx.enter_context(tc.tile_pool(name="st", bufs=6))
        psf_pool = ctx.enter_context(tc.tile_pool(name="psf", bufs=2, space="PSUM"))
        psb_pool = ctx.enter_context(tc.tile_pool(name="psb", bufs=6, space="PSUM"))

        identity = const_pool.tile([128, 128], F16)
        nc.scalar.dma_start(identity[:], iden.ap())
        bias_t = const_pool.tile([128, R], F32)
        nc.scalar.dma_start(bias_t[:], bias.ap())
        # Constant NEG source for halo pads (ACT copies cast f32->f16 where
        # the destination tile is fp16; -10000 is exactly representable).
        neg_t = const_pool.tile([128, NS * EL], F32)
        nc.gpsimd.memset(neg_t[:], NEG)

        def set_pads(tile_, seg):
            v = tile_[:].rearrange("p (s c) -> p s c", s=NS)
            nv = neg_t[:].rearrange("p (s c) -> p s c", s=NS)
            nc.scalar.copy(v[:, :, 0:EL], nv)
            nc.scalar.copy(v[:, :, seg - ER : seg], nv[:, :, 0:ER])

        for _rep in range(reps):
          prev_haccs = None
          for g in range(nG + 1):
            haccs = []
            if g < nG:
                # ---- horizontal pass over nHB batched row-tiles ----
                for b in range(nHB):
                    ht = hin_pool.tile([128, NS * SW], F32, tag="hin")
                    set_pads(ht, SW)
                    for tl in range(TB):
                        for ci in range(CP):
                            s0 = (tl * CP + ci) * SW
                            t = b * TB + tl
                            # spread load dispatch across both DGE queues
                            ld_eng = nc.sync if (tl * CP + ci) % 2 == 0 else nc.gpsimd
                            ld_eng.dma_start(
                                ht[:, s0 + EL : s0 + EL + W],
                                im.ap()[g * CP + ci, t * 128 : (t + 1) * 128, :],
                            )
                    acc = hacc_pool.tile([128, NS * AW], F16, tag="hacc")
                    accv = acc[:].rearrange("p (s c) -> p s c", s=NS)
                    src3 = ht[:].rearrange("p (s c) -> p s c", s=NS)
                    if g == 0 and b == 0:
                        # warm-up: first segment separately so the first DIL3
                        # starts after one channel's DMA
                        _cascade(nc, hmid_pool, src3[:, 0:1, :], bias_t,
                                 SW, W, accv[:, 0:1, :], tag="m1a", bufs=2)
                        _cascade(nc, hmid_pool, src3[:, 1:CP, :], bias_t,
                                 SW, W, accv[:, 1:CP, :], tag="m1c", bufs=2)
                        _cascade(nc, hmid_pool, src3[:, CP:NS, :], bias_t,
                                 SW, W, accv[:, CP:NS, :], tag="m1d", bufs=2)
                    else:
                        _cascade(nc, hmid_pool, src3, bias_t, SW, W, accv)
                    haccs.append(acc)

            if prev_haccs is not None:
                pg = g - 1
                # ---- transpose + vertical pass over nWB batched col-tiles ----
                vaccs = []
                for vb in range(nWB):
                    vt = vin_pool.tile([128, NS * SH], F16, tag="vin")
                    set_pads(vt, SH)
                    for wl in range(TB):
                        w = vb * TB + wl
                        for ci in range(CP):
                            pt = psf_pool.tile([128, H], F16, tag="psf")
                            for t in range(nH):
                                hb, tl = divmod(t, TB)
                                nc.tensor.transpose(
                                    pt[:, t * 128 : (t + 1) * 128],
                                    prev_haccs[hb][
                                        :,
                                        (tl * CP + ci) * AW + JW + w * 128 :
                                        (tl * CP + ci) * AW + JW + (w + 1) * 128,
                                    ],
                                    identity[:],
                                )
                            s0 = (wl * CP + ci) * SH
                            nc.scalar.copy(vt[:, s0 + EL : s0 + EL + H], pt[:])
                    vacc = vacc_pool.tile([128, NS * AH], F16, tag="vacc")
                    vaccv = vacc[:].rearrange("p (s c) -> p s c", s=NS)
                    vsrc3 = vt[:].rearrange("p (s c) -> p s c", s=NS)
                    _cascade(nc, vmid_pool, vsrc3, bias_t, SH, H, vaccv)
                    vaccs.append(vacc)

                # ---- transpose back + store (chunked; DMA dispatch
                # alternates between the SP and Pool sequencers) ----
                for vb2 in range(nWB):
                    for ci in range(CP):
                        for t in range(nH):
                            qt = psb_pool.tile([128, TB * 128], F16, tag="psb")
                            for wl in range(TB):
                                nc.tensor.transpose(
                                    qt[:, wl * 128 : (wl + 1) * 128],
                                    vaccs[vb2][
                                        :,
                                        (wl * CP + ci) * AH + JW + t * 128 :
                                        (wl * CP + ci) * AH + JW + (t + 1) * 128,
                                    ],
                                    identity[:],
                                )
                            st = st_pool.tile([128, TB * 128], F32, tag="st")
                            nc.scalar.copy(st[:], qt[:])
                            if pg == nG - 1:
                                # final group: 3-way dispatch; ACT's queue is
                                # nearly drained during the last cascades
                                dma_engines = (nc.sync, nc.gpsimd, nc.scalar)
                                dma_eng = dma_engines[(ci * nH + t) % 3]
                            else:
                                dma_eng = (
                                    nc.sync if (ci * nH + t) % 2 == 0 else nc.gpsimd
                                )
                            dma_eng.dma_start(
                                out.ap()[
                                    pg * CP + ci,
                                    t * 128 : (t + 1) * 128,
                                    vb2 * TB * 128 : (vb2 + 1) * TB * 128,
                                ],
                                st[:],
                            )
            prev_haccs = haccs if g < nG else None

    nc.compile()
    return nc


_NC_CACHE = {}


def _get_nc():
    if "nc" not in _NC_CACHE:
        _NC_CACHE["nc"] = build_nc()
    return _NC_CACHE["nc"]


def _make_in_maps(im, se_coef, se):
    im = np.ascontiguousarray(np.asarray(im, dtype=np.float32))
    se = np.asarray(se, dtype=np.float32)
    se_coef = np.asarray(se_coef, dtype=np.float32)
    a = (se_coef * se[R + 1, 0]).astype(np.float32)  # a = se_coef/4 (exact)
    cs = (a * np.arange(1, 2 * R, 2, dtype=np.float32)).astype(np.float32)
    bias5 = np.ascontiguousarray(np.broadcast_to(cs, (128, R))).astype(np.float32)
    iden = np.eye(128, dtype=np.float16)
    return [
        {"im": im[b], "bias5": bias5, "iden": iden} for b in range(im.shape[0])
    ]


def kernel(im, se_coef, se):
    nc = _get_nc()
    in_maps = _make_in_maps(im, se_coef, se)
    res = run_bass_kernel_spmd(nc, in_maps, core_ids=list(range(N_CORES)))
    out = np.stack([res.results[b]["out"] for b in range(N_CORES)], axis=0)
    return out.astype(np.float32)


# revision 41
# speedup vs baseline: 1.0005x; 1.0005x over previous
"""Trainium2 Bass kernel: separable parabolic morphological dilation (11-tap).

nn_Dilation2dSingle: im [8, 32, 512, 512] f32, se_coef scalar, se [11, 1].
    bias[k] = se_coef * se[k, 0] = a * k^2,  a = se_coef / 4
    out = vdilate(hdilate(im)) with NEG=-10000 padding.

The 11-tap parabolic window is decomposed exactly into a cascade of five
3-tap max-plus stages with biases c_m = a*(2m-1) (partial sums a*k^2):
    u_m[j] = max(u_{m-1}[j], max(u_{m-1}[j-1], u_{m-1}[j+1]) - c_m)
Each stage runs as ONE custom DVE instruction (DIL3_ANT): a hand-built
uop program recovers the center tap as a one-element temporal delay of
SRC_0 inside the 8-slice pipeline (block 0 captures its previous-cycle
ALU flop into a delay lane), so a stage costs 1 elem/cycle instead of
the 2 ops/elem of the stock tensor_max + scalar_tensor_tensor pair.
DVE work: 5 passes/axis instead of 10 -> ~2x.

Each DIL3 stream's first element (and each segment-boundary first
element) computes with a stale delayed tap. Stage m's garbage cell sits
at position m and is only ever read by stage m+1's own first element --
itself garbage -- so the junk chain is self-contained; the left halo is
widened to 6 so the final stage's garbage lands in a never-read leading
column of the accumulator tile.

Two row/col-tiles are batched per DVE instruction (8 segments per 3D
access pattern) to halve instruction-dispatch overhead; intermediates
are fp16 to fit the batched tiles in SBUF (NEG is exactly representable
in fp16; ~5e-4 relative rounding vs the 2e-2 gate). Input/output stay
fp32.

Sharding: pure data-parallel over batch (8 cores x 1 batch each); no
collectives.
"""

from contextlib import ExitStack
from dataclasses import dataclass

import numpy as np

import concourse.bacc as bacc
import concourse.dve_ops as dve_ops_mod
import concourse.mybir as mybir
import concourse.tile as tile
from concourse.bass_utils import run_bass_kernel_spmd
from concourse.dve_ops import DveOp
from concourse.dve_spec import C0, Spec, Src0, Src1, maxx
from concourse.dve_uop import (
    ENABLE,
    AluInp,
    AluOp,
    DelayInp,
    DveOpSpec,
    InpSel,
    OutPath,
    OutSel,
    Trigger,
    UopConfig,
)

F32 = mybir.dt.float32
F16 = mybir.dt.float16
NEG = -10000.0
R = 5  # dilation radius (window 11)
EL, ER = 6, 5  # left/right halo widths (left widened for the junk cell)
JW = 1  # junk leading column on the accumulator tiles

# Hardcoded problem shape (per spec).
B, C, H, W = 8, 32, 512, 512
N_CORES = 8
CP = 4  # channels per group
TB = 2  # row/col tiles batched per DVE instruction
NS = CP * TB  # segments per DVE instruction


# --- DIL3_ANT custom DVE op ------------------------------------------------ #


def _build_dil3_uops() -> list[UopConfig]:
    """out[j] = max(x[j], max(x[j-1], x[j+1]) - c) with in0 = x[j+1],
    in1 = x[j-1]; the center x[j] is block 0's previous-cycle flop."""
    u = UopConfig()
    u.enable_input(InpSel.SRC_0, 1)  # delay_0 <- x[j+1]
    u.enable_input(InpSel.SRC_1, 2)  # delay_1 <- x[j-1]
    u.enable_input(InpSel.CONST_0, 3)  # delay_2 <- c
    dp = u.datapath_config
    dp[0].enable_alu(AluOp.BYPASS, AluInp.PREV_DELAY_0)
    dp[0].enable_delay_from_src(DelayInp.CURR_ALU_OUT, 3)  # delay_3 <- x[j]
    dp[0].pass_through_delay(1, 2)
    dp[1].enable_alu(AluOp.MAX, AluInp.PREV_ALU_OUT, AluInp.PREV_DELAY_1)
    dp[1].pass_through_delay(2, 3)
    dp[2].enable_alu(AluOp.SUBTRACT, AluInp.PREV_ALU_OUT, AluInp.PREV_DELAY_2)
    dp[2].pass_through_delay(3)
    dp[3].enable_alu(AluOp.MAX, AluInp.PREV_ALU_OUT, AluInp.PREV_DELAY_3)
    for k in range(4, 8):
        dp[k].pass_through_alu()
    u.require_inp0 = ENABLE
    u.require_inp1 = ENABLE
    u.trigger = (Trigger.SRC_TENSOR_DONE, Trigger.NONE, Trigger.NONE)
    u.enable_output(OutSel.ALU_OUT, OutPath.WR0_LO)
    return [u]


_HAND_CACHE: dict = {}


@dataclass(frozen=True)
class HandDveOp(DveOp):
    """DveOp whose table program is hand-built rather than lowered from
    `spec`; `spec` is only the structural stand-in for _custom_dve."""

    def compile(self, ver):
        key = (self.name, ver)
        if key not in _HAND_CACHE:
            s = DveOpSpec(
                name=self.name,
                opcode=dve_ops_mod.get_dve_sub_opcode(self.name),
                uops=_build_dil3_uops(),
                rd1_en=True,
            )
            s.validate(ver)
            _HAND_CACHE[key] = s
        return _HAND_CACHE[key]


def _dil3_ref(in0, in1, s0, s1, imm2):
    return np.maximum(in1, in0 - s0).astype(np.float32)


DIL3 = HandDveOp(
    "DIL3_ANT",
    Spec(body=maxx(Src1, Src0 - C0), reference=_dil3_ref),
    subdim=False,
    uops_sha={},
)


def register_dil3() -> None:
    if DIL3.name in dve_ops_mod._SUB_OPCODE_FOR_NAME:
        return
    row = dve_ops_mod._CUSTOM_DVE_ROW_BASE + len(dve_ops_mod.OPS)
    assert row < 0x20, f"no free custom-DVE row for {DIL3.name}"
    dve_ops_mod.OPS.append(DIL3)
    dve_ops_mod._SUB_OPCODE_FOR_NAME[DIL3.name] = row
    dve_ops_mod.CUSTOM_DVE_SPECS[DIL3.name] = DIL3.spec


register_dil3()


# --- kernel ---------------------------------------------------------------- #


def _cascade(nc, mid_pool, seg3, bias_t, S, L, acc3, tag="mid", bufs=None):
    """Five DIL3 stages along the innermost axis of seg3 [128, nseg, S]
    (positions: [0,EL) NEG pad, [EL,EL+L) payload, [EL+L,S) NEG pad).
    Writes acc3 [128, nseg, L+JW]; valid payload at [JW, JW+L).
    Intermediates are fp16 (values +-~30; NEG is exact in fp16; the
    ~5e-4 relative rounding is far inside the 2e-2 gate)."""
    prev = seg3
    for m in range(1, R):
        kw = {} if bufs is None else {"bufs": bufs}
        mid = mid_pool.tile([128, seg3.shape[1] * S], F16, tag=tag, **kw)
        midv = mid[:].rearrange("p (s c) -> p s c", s=seg3.shape[1])
        nc.vector._custom_dve(
            DIL3,
            out=midv[:, :, m : S - m],
            in0=prev[:, :, m + 1 : S - m + 1],
            in1=prev[:, :, m - 1 : S - m - 1],
            s0=bias_t[:, m - 1 : m],
        )
        prev = midv
    nc.vector._custom_dve(
        DIL3,
        out=acc3[:, :, 0 : L + JW],
        in0=prev[:, :, R + 1 : S - R + 1],
        in1=prev[:, :, R - 1 : S - R - 1],
        s0=bias_t[:, R - 1 : R],
    )


def build_nc(C=C, H=H, W=W, CP=CP, reps=1):
    assert H % 128 == 0 and W % 128 == 0 and C % CP == 0
    nH, nW, nG = H // 128, W // 128, C // CP
    nHB, nWB = nH // TB, nW // TB  # batched tile counts
    SW, SH = W + EL + ER, H + EL + ER
    AW, AH = W + JW, H + JW  # accumulator widths per segment

    nc = bacc.Bacc("TRN2", target_bir_lowering=False, debug=False)
    im = nc.dram_tensor("im", [C, H, W], F32, kind="ExternalInput")
    bias = nc.dram_tensor("bias5", [128, R], F32, kind="ExternalInput")
    iden = nc.dram_tensor("iden", [128, 128], F16, kind="ExternalInput")
    out = nc.dram_tensor("out", [C, H, W], F32, kind="ExternalOutput")

    with tile.TileContext(nc) as tc, ExitStack() as ctx:
        const_pool = ctx.enter_context(tc.tile_pool(name="const", bufs=1))
        hin_pool = ctx.enter_context(tc.tile_pool(name="hin", bufs=3))
        hmid_pool = ctx.enter_context(tc.tile_pool(name="hmid", bufs=2))
        hacc_pool = ctx.enter_context(tc.tile_pool(name="hacc", bufs=2 * nHB + 1))
        vin_pool = ctx.enter_context(tc.tile_pool(name="vin", bufs=3))
        vmid_pool = ctx.enter_context(tc.tile_pool(name="vmid", bufs=3))
        vacc_pool = ctx.enter_context(tc.tile_pool(name="vacc", bufs=nWB + 1))
        st_pool = ctx.enter_context(tc.tile_pool(name="st", bufs=8))
        psf_pool = ctx.enter_context(tc.tile_pool(name="psf", bufs=2, space="PSUM"))
        psb_pool = ctx.enter_context(tc.tile_pool(name="psb", bufs=6, space="PSUM"))

        identity = const_pool.tile([128, 128], F16)
        nc.scalar.dma_start(identity[:], iden.ap())
        bias_t = const_pool.tile([128, R], F32)
        nc.scalar.dma_start(bias_t[:], bias.ap())
        # Constant NEG source for halo pads (ACT copies cast f32->f16 where
        # the destination tile is fp16; -10000 is exactly representable).
        neg_t = const_pool.tile([128, NS * EL], F32)
        nc.gpsimd.memset(neg_t[:], NEG)

        def set_pads(tile_, seg):
            v = tile_[:].rearrange("p (s c) -> p s c", s=NS)
            nv = neg_t[:].rearrange("p (s c) -> p s c", s=NS)
            nc.scalar.copy(v[:, :, 0:EL], nv)
            nc.scalar.copy(v[:, :, seg - ER : seg], nv[:, :, 0:ER])

        for _rep in range(reps):
          prev_haccs = None
          for g in range(nG + 1):
            haccs = []
            if g < nG:
                # ---- horizontal pass over nHB batched row-tiles ----
                for b in range(nHB):
                    ht = hin_pool.tile([128, NS * SW], F32, tag="hin")
                    set_pads(ht, SW)
                    for tl in range(TB):
                        for ci in range(CP):
                            s0 = (tl * CP + ci) * SW
                            t = b * TB + tl
                            # spread load dispatch across both DGE queues
                            ld_eng = nc.sync if (tl * CP + ci) % 2 == 0 else nc.gpsimd
                            ld_eng.dma_start(
                                ht[:, s0 + EL : s0 + EL + W],
                                im.ap()[g * CP + ci, t * 128 : (t + 1) * 128, :],
                            )
                    acc = hacc_pool.tile([128, NS * AW], F16, tag="hacc")
                    accv = acc[:].rearrange("p (s c) -> p s c", s=NS)
                    src3 = ht[:].rearrange("p (s c) -> p s c", s=NS)
                    if g == 0 and b == 0:
                        # warm-up: first segment separately so the first DIL3
                        # starts after one channel's DMA
                        _cascade(nc, hmid_pool, src3[:, 0:1, :], bias_t,
                                 SW, W, accv[:, 0:1, :], tag="m1a", bufs=2)
                        _cascade(nc, hmid_pool, src3[:, 1:CP, :], bias_t,
                                 SW, W, accv[:, 1:CP, :], tag="m1c", bufs=2)
                        _cascade(nc, hmid_pool, src3[:, CP:NS, :], bias_t,
                                 SW, W, accv[:, CP:NS, :], tag="m1d", bufs=2)
                    else:
                        _cascade(nc, hmid_pool, src3, bias_t, SW, W, accv)
                    haccs.append(acc)

            if prev_haccs is not None:
                pg = g - 1
                # ---- transpose + vertical pass over nWB batched col-tiles ----
                vaccs = []
                for vb in range(nWB):
                    vt = vin_pool.tile([128, NS * SH], F16, tag="vin")
                    set_pads(vt, SH)
                    for wl in range(TB):
                        w = vb * TB + wl
                        for ci in range(CP):
                            pt = psf_pool.tile([128, H], F16, tag="psf")
                            for t in range(nH):
                                hb, tl = divmod(t, TB)
                                nc.tensor.transpose(
                                    pt[:, t * 128 : (t + 1) * 128],
                                    prev_haccs[hb][
                                        :,
                                        (tl * CP + ci) * AW + JW + w * 128 :
                                        (tl * CP + ci) * AW + JW + (w + 1) * 128,
                                    ],
                                    identity[:],
                                )
                            s0 = (wl * CP + ci) * SH
                            nc.scalar.copy(vt[:, s0 + EL : s0 + EL + H], pt[:])
                    vacc = vacc_pool.tile([128, NS * AH], F16, tag="vacc")
                    vaccv = vacc[:].rearrange("p (s c) -> p s c", s=NS)
                    vsrc3 = vt[:].rearrange("p (s c) -> p s c", s=NS)
                    _cascade(nc, vmid_pool, vsrc3, bias_t, SH, H, vaccv)
                    vaccs.append(vacc)

                # ---- transpose back + store (chunked; DMA dispatch
                # alternates between the SP and Pool sequencers) ----
                for vb2 in range(nWB):
                    for ci in range(CP):
                        for t in range(nH):
                            qt = psb_pool.tile([128, TB * 128], F16, tag="psb")
                            for wl in range(TB):
                                nc.tensor.transpose(
                                    qt[:, wl * 128 : (wl + 1) * 128],
                                    vaccs[vb2][
                                        :,
                                        (wl * CP + ci) * AH + JW + t * 128 :
                                        (wl * CP + ci) * AH + JW + (t + 1) * 128,
                                    ],
                                    identity[:],
                                )
                            st = st_pool.tile([128, TB * 128], F32, tag="st")
                            nc.scalar.copy(st[:], qt[:])
                            if pg == nG - 1:
                                # final group: 3-way dispatch; ACT's queue is
                                # nearly drained during the last cascades
                                dma_engines = (nc.sync, nc.gpsimd, nc.scalar)
                                dma_eng = dma_engines[(ci * nH + t) % 3]
                            else:
                                dma_eng = (
                                    nc.sync if (ci * nH + t) % 2 == 0 else nc.gpsimd
                                )
                            dma_eng.dma_start(
                                out.ap()[
                                    pg * CP + ci,
                                    t * 128 : (t + 1) * 128,
                                    vb2 * TB * 128 : (vb2 + 1) * TB * 128,
                                ],
                                st[:],
                            )
            prev_haccs = haccs if g < nG else None

    nc.compile()
    return nc


_NC_CACHE = {}


def _get_nc():
    if "nc" not in _NC_CACHE:
        _NC_CACHE["nc"] = build_nc()
    return _NC_CACHE["nc"]


def _make_in_maps(im, se_coef, se):
    im = np.ascontiguousarray(np.asarray(im, dtype=np.float32))
    se = np.asarray(se, dtype=np.float32)
    se_coef = np.asarray(se_coef, dtype=np.float32)
    a = (se_coef * se[R + 1, 0]).astype(np.float32)  # a = se_coef/4 (exact)
    cs = (a * np.arange(1, 2 * R, 2, dtype=np.float32)).astype(np.float32)
    bias5 = np.ascontiguousarray(np.broadcast_to(cs, (128, R))).astype(np.float32)
    iden = np.eye(128, dtype=np.float16)
    return [
        {"im": im[b], "bias5": bias5, "iden": iden} for b in range(im.shape[0])
    ]


def kernel(im, se_coef, se):
    nc = _get_nc()
    in_maps = _make_in_maps(im, se_coef, se)
    res = run_bass_kernel_spmd(nc, in_maps, core_ids=list(range(N_CORES)))
    out = np.stack([res.results[b]["out"] for b in range(N_CORES)], axis=0)
    return out.astype(np.float32)
